# revision 3
# baseline (speedup 1.0000x reference)
"""Trainium2 Bass kernel for nn_Domain_Attention.

Reference computation (B=128, N=196, C=768, centers=64):
    q    = x.mean(1)                                  (B, C)
    attn = softmax(l2n(q) @ l2n(k_c) * C**-0.5)       (B, 64)
    v    = einsum('bc,cnm->bnm', attn, v_c)           (B, N, N)
    out  = einsum('bnm,bmd->bnd', v, x) @ W.T + b     (B, N, C)

Strategy: data-parallel over B across 8 NeuronCores (16 batches/core),
k_c / v_c / proj weights replicated.  Per core the computation is
reassociated as out[b] = v[b] @ (x[b] @ W.T) + b so the big GEMMs keep
the contraction dim on SBUF partitions with no on-chip transposes of
large tensors; the only cross-partition move is the (16, 38416) mixture
result -> per-m-row vT tiles, done with SBUF->SBUF DMAs of 784B runs.

Matmuls run as float32r (full-rate fp32 path at moving-dim >= 256).
"""

import numpy as np

import concourse.bass as bass
import concourse.mybir as mybir
import concourse.tile as tile
from concourse import bacc
from concourse.bass_utils import run_bass_kernel_spmd
from concourse.masks import make_identity

F32 = mybir.dt.float32
F32R = mybir.dt.float32r
AX = mybir.AxisListType
ALU = mybir.AluOpType
ACT_F = mybir.ActivationFunctionType

N_CORES = 8
B, N, C, CEN = 128, 196, 768, 64
BL = B // N_CORES            # 16 local batches
D6 = C // 128                # 6 d-chunks
MB = 2 * N                   # 392 x-block columns = 2 batches
NBLK = BL // 2               # 8 blocks
ECH = [(0, 384), (384, 384)]             # e-chunks for 768
MCH = [(0, 128), (128, N - 128)]         # m-chunks per batch: 128 + 68
HALF = N * N // 2            # 19208 flat (m,i) elements per half (98 m-rows)
PCW = 7 * MB                 # 2744 vct piece width (7 m-pair chunks)
NPC = HALF // PCW            # 7 pieces
P2_START = 8                 # batches >= this get z computed after the mixture


def _r(ap):
    return ap.bitcast(F32R)


def build_nc():
    nc = bacc.Bacc("TRN2", target_bir_lowering=False, debug=False)
    xt_d = nc.dram_tensor("xt", [C, BL * N], F32R, kind="ExternalInput").ap()
    kc_d = nc.dram_tensor("kc", [C, CEN], F32, kind="ExternalInput").ap()
    vct_d = nc.dram_tensor("vct", [128, HALF], F32R, kind="ExternalInput").ap()
    wt_d = nc.dram_tensor("wt", [C, C], F32R, kind="ExternalInput").ap()
    pb_d = nc.dram_tensor("pb", [128, C], F32, kind="ExternalInput").ap()
    out_d = nc.dram_tensor("out", [BL * N, C], F32, kind="ExternalOutput").ap()

    with tile.TileContext(nc) as tc:
        with (
            tc.tile_pool(name="consts", bufs=1) as consts,
            tc.tile_pool(name="wtp", bufs=D6) as wtp,
            tc.tile_pool(name="kcp", bufs=D6) as kcp,
            tc.tile_pool(name="xtp", bufs=30) as xtp,
            tc.tile_pool(name="vctp", bufs=2) as vctp,
            tc.tile_pool(name="zp", bufs=9) as zp,
            tc.tile_pool(name="vtp", bufs=1) as vtp,
            tc.tile_pool(name="smallp", bufs=2) as smallp,
            tc.tile_pool(name="stagep", bufs=4) as stagep,
            tc.tile_pool(name="outp", bufs=4) as outp,
            tc.tile_pool(name="ps_z", bufs=3, space="PSUM") as ps_z,
        ):
            ones = consts.tile([128, 1], F32)
            nc.vector.memset(ones[:], 1.0)
            ident = consts.tile([128, 128], F32)
            make_identity(nc, ident[:])
            pb_t = consts.tile([128, C], F32)
            nc.sync.dma_start(pb_t[:], pb_d[:])
            qt_all = consts.tile([128, D6 * BL], F32)   # per d-chunk: 16 batch sums
            vT = vtp.tile([128, BL * 2 * N], F32R)       # [m%128][b, m//128, i]

            wt_t = []
            for d in range(D6):
                t = wtp.tile([128, C], F32R, tag="wt")
                nc.sync.dma_start(t[:], wt_d[128 * d:128 * (d + 1), :])
                wt_t.append(t)
            kc_t = []
            for d in range(D6):
                t = kcp.tile([128, CEN], F32, tag="kc")
                nc.sync.dma_start(t[:], kc_d[128 * d:128 * (d + 1), :])
                kc_t.append(t)

            z_t = [None] * BL

            def z_batch(b, xb):
                """z[b] = x[b] @ W.T as (128 part m, half, 768 e), from block tiles."""
                lb = b % 2
                zt = zp.tile([128, 2 * C], F32R, tag="z")
                z_t[b] = zt
                for mi, (moff, msz) in enumerate(MCH):
                    pse = [ps_z.tile([128, 384], F32, tag="psz", name=f"psz{e}") for e in range(2)]
                    for d in range(D6):
                        lhsT = xb[d][:, lb * N + moff: lb * N + moff + msz]
                        for e, (eoff, esz) in enumerate(ECH):
                            nc.tensor.matmul(
                                pse[e][:msz, :esz],
                                lhsT=lhsT,
                                rhs=wt_t[d][:, eoff:eoff + esz],
                                start=(d == 0), stop=(d == D6 - 1),
                            )
                    for e, (eoff, esz) in enumerate(ECH):
                        nc.scalar.copy(
                            zt[:msz, mi * C + eoff: mi * C + eoff + esz],
                            pse[e][:msz, :esz],
                        )

            # ---- phase B: x loads, q reduction, z for batches < P2_START ----
            xb_t = []
            for k in range(NBLK):
                xb = []
                for d in range(D6):
                    t = xtp.tile([128, MB], F32R, tag="xb")
                    nc.sync.dma_start(t[:], xt_d[128 * d:128 * (d + 1),
                                                 MB * k: MB * (k + 1)])
                    xb.append(t)
                xb_t.append(xb)
                for d in range(D6):
                    nc.vector.tensor_reduce(
                        out=qt_all[:, d * BL + 2 * k: d * BL + 2 * k + 2],
                        in_=xb[d].bitcast(F32).rearrange("p (t n) -> p t n", n=N),
                        axis=AX.X, op=ALU.add,
                    )
                if 2 * k + 1 < P2_START:
                    z_batch(2 * k, xb)
                    z_batch(2 * k + 1, xb)

            # ---- phase C: attention weights ----
            with tc.tile_pool(name="ps_c", bufs=2, space="PSUM") as ps_c:
                qsq = smallp.tile([128, D6 * BL], F32, tag="qsq")
                nc.scalar.square(qsq[:], qt_all[:])
                ssqq = ps_c.tile([BL, 1], F32, tag="psc1")
                for d in range(D6):
                    nc.tensor.matmul(ssqq[:], lhsT=qsq[:, d * BL:(d + 1) * BL],
                                     rhs=ones[:], start=(d == 0), stop=(d == D6 - 1))
                rq1 = smallp.tile([BL, 1], F32, tag="rq1")
                nc.vector.reciprocal(rq1[:], ssqq[:])
                rs_q = smallp.tile([BL, 1], F32, tag="rsq")
                nc.scalar.activation(rs_q[:], rq1[:], ACT_F.Sqrt, scale=1.0 / C)

                ksq = smallp.tile([128, CEN], F32, tag="ksq")
                ssqk = ps_c.tile([CEN, 1], F32, tag="psc1")
                for d in range(D6):
                    nc.scalar.square(ksq[:], kc_t[d][:])
                    nc.tensor.matmul(ssqk[:], lhsT=ksq[:], rhs=ones[:],
                                     start=(d == 0), stop=(d == D6 - 1))
                rk1 = smallp.tile([CEN, 1], F32, tag="rk1")
                nc.vector.reciprocal(rk1[:], ssqk[:])
                rt = smallp.tile([CEN, 1], F32, tag="rt")
                nc.scalar.sqrt(rt[:], rk1[:])

                lgT = ps_c.tile([CEN, BL], F32, tag="psc1")
                for d in range(D6):
                    nc.tensor.matmul(lgT[:], lhsT=kc_t[d][:],
                                     rhs=qt_all[:, d * BL:(d + 1) * BL],
                                     start=(d == 0), stop=(d == D6 - 1))
                lgT_s = smallp.tile([CEN, BL], F32, tag="lgts")
                nc.vector.tensor_scalar_mul(lgT_s[:], lgT[:], rt[:])

                lg = ps_c.tile([BL, CEN], F32, tag="psc2")
                nc.tensor.transpose(lg[:], lgT_s[:], ident[:CEN, :CEN])
                lg2 = smallp.tile([BL, CEN], F32, tag="lg2")
                nc.vector.tensor_scalar_mul(lg2[:], lg[:], rs_q[:])

                mx = smallp.tile([BL, 1], F32, tag="mx")
                nc.vector.tensor_reduce(out=mx[:], in_=lg2[:], axis=AX.X, op=ALU.max)
                nmx = smallp.tile([BL, 1], F32, tag="nmx")
                nc.vector.tensor_scalar_mul(nmx[:], mx[:], -1.0)
                ex = smallp.tile([BL, CEN], F32, tag="ex")
                sm = smallp.tile([BL, 1], F32, tag="sm")
                nc.scalar.activation(ex[:], lg2[:], ACT_F.Exp, bias=nmx[:],
                                     accum_out=sm[:])
                rsm = smallp.tile([BL, 1], F32, tag="rsm")
                nc.vector.reciprocal(rsm[:], sm[:])
                attn = smallp.tile([BL, CEN], F32, tag="attn")
                nc.vector.tensor_scalar_mul(attn[:], ex[:], rsm[:])

                attnT_ps = ps_c.tile([CEN, BL], F32, tag="psc1")
                nc.tensor.transpose(attnT_ps[:], attn[:], ident[:BL, :BL])
                attnT = smallp.tile([128, BL], F32R, tag="attnT")
                nc.scalar.copy(attnT[:CEN, :], attnT_ps[:])
                nc.sync.dma_start(attnT[CEN:128, :], attnT[:CEN, :])

            # ---- phase D: mixture vT[b][m, i] + partition shuffle ----
            with tc.tile_pool(name="ps_mix", bufs=3, space="PSUM") as ps_mix:
                for p in range(NPC):
                    vc = vctp.tile([128, PCW], F32R, tag="vct")
                    nc.sync.dma_start(vc[:], vct_d[:, PCW * p: PCW * (p + 1)])
                    for j in range(PCW // MB):
                        cj = p * (PCW // MB) + j
                        for h in range(2):
                            ps = ps_mix.tile([BL, MB], F32, tag="psmix")
                            nc.tensor.matmul(
                                ps[:],
                                lhsT=attnT[64 * h: 64 * h + CEN, :],
                                rhs=vc[64 * h: 64 * h + CEN,
                                       MB * j: MB * (j + 1)],
                                start=True, stop=True,
                            )
                            st = stagep.tile([BL, MB], F32R, tag="stage")
                            if h == 0:
                                nc.vector.tensor_copy(st[:], ps[:])
                            else:
                                nc.scalar.copy(st[:], ps[:])
                            m0 = 2 * cj + 98 * h
                            for mp in range(2):
                                m = m0 + mp
                                part, half = (m, 0) if m < 128 else (m - 128, 1)
                                dst = vT[part:part + 1, :].rearrange(
                                    "p (b t i) -> p b t i", b=BL, t=2)[:, :, half, :]
                                nc.sync.dma_start(dst, st[:, N * mp: N * (mp + 1)])

            # ---- final matmuls for first-half batches, then deferred z ----
            def final_batch(b):
                with tc.tile_pool(name=f"ps_f{b}", bufs=4, space="PSUM") as ps_f:
                    zt = z_t[b]
                    for ioff, isz in MCH:
                        pse = [ps_f.tile([128, 384], F32, tag="psf", name=f"psf{e}") for e in range(2)]
                        for half, (_, ksz) in enumerate(MCH):
                            lhsT = vT[:ksz].rearrange(
                                "p (b t i) -> p b t i", b=BL, t=2)[
                                :, b, half, ioff:ioff + isz]
                            for e, (eoff, esz) in enumerate(ECH):
                                nc.tensor.matmul(
                                    pse[e][:isz, :esz],
                                    lhsT=lhsT,
                                    rhs=zt[:ksz, half * C + eoff:
                                             half * C + eoff + esz],
                                    start=(half == 0), stop=(half == 1),
                                )
                        for e, (eoff, esz) in enumerate(ECH):
                            ot = outp.tile([128, 384], F32, tag="ot")
                            nc.vector.tensor_tensor(
                                out=ot[:isz, :esz], in0=pse[e][:isz, :esz],
                                in1=pb_t[:isz, eoff:eoff + esz], op=ALU.add)
                            nc.sync.dma_start(
                                out_d[b * N + ioff: b * N + ioff + isz,
                                      eoff:eoff + esz],
                                ot[:isz, :esz])

            for b in range(P2_START):
                final_batch(b)
            for b in range(P2_START, BL):
                z_batch(b, xb_t[b // 2])
            for b in range(P2_START, BL):
                final_batch(b)

    nc.compile()
    return nc


_NC = None


def _get_nc():
    global _NC
    if _NC is None:
        _NC = build_nc()
    return _NC


def kernel(x, k_c, v_c, proj_w, proj_b):
    x = np.ascontiguousarray(x, dtype=np.float32)
    k_c = np.ascontiguousarray(k_c, dtype=np.float32)
    v_c = np.ascontiguousarray(v_c, dtype=np.float32)
    proj_w = np.ascontiguousarray(proj_w, dtype=np.float32)
    proj_b = np.ascontiguousarray(proj_b, dtype=np.float32)

    # replicated, layout-marshalled operands
    vt = v_c.transpose(0, 2, 1).reshape(CEN, N * N)      # [c, m*196+i]
    vct = np.concatenate([vt[:, :HALF], vt[:, HALF:]], axis=0)  # (128, 19208)
    vct = np.ascontiguousarray(vct)
    wt = np.ascontiguousarray(proj_w.T)                  # wt[d, e] = proj_w[e, d]
    pb = np.ascontiguousarray(np.broadcast_to(proj_b, (128, C)))

    in_maps = []
    for c in range(N_CORES):
        xs = x[BL * c: BL * (c + 1)].reshape(BL * N, C)
        xt = np.ascontiguousarray(xs.T)                  # (768, 3136)
        in_maps.append({"xt": xt, "kc": k_c, "vct": vct, "wt": wt, "pb": pb})

    nc = _get_nc()
    res = run_bass_kernel_spmd(nc, in_maps, list(range(N_CORES)))
    out = np.concatenate(
        [r["out"].reshape(BL, N, C) for r in res.results], axis=0)
    return out


# revision 4
# speedup vs baseline: 1.2710x; 1.2710x over previous
"""Trainium2 Bass kernel for nn_Domain_Attention.

Reference computation (B=128, N=196, C=768, centers=64):
    q    = x.mean(1)                                  (B, C)
    attn = softmax(l2n(q) @ l2n(k_c) * C**-0.5)       (B, 64)
    v    = einsum('bc,cnm->bnm', attn, v_c)           (B, N, N)
    out  = einsum('bnm,bmd->bnd', v, x) @ W.T + b     (B, N, C)

Strategy: data-parallel over B across 8 NeuronCores (16 batches/core),
k_c / v_c / proj weights replicated.  Per core the computation is
reassociated as out[b] = v[b] @ (x[b] @ W.T) + b so the big GEMMs keep
the contraction dim on SBUF partitions with no on-chip transposes of
large tensors; the only cross-partition move is the (16, 38416) mixture
result -> per-m-row vT tiles, done with SBUF->SBUF DMAs of 784B runs.

Matmuls run as float32r (full-rate fp32 path at moving-dim >= 256).
"""

import numpy as np

import concourse.bass as bass
import concourse.mybir as mybir
import concourse.tile as tile
from concourse import bacc
from concourse.bass_utils import run_bass_kernel_spmd
from concourse.masks import make_identity

F32 = mybir.dt.float32
F32R = mybir.dt.float32r
AX = mybir.AxisListType
ALU = mybir.AluOpType
ACT_F = mybir.ActivationFunctionType

N_CORES = 8
B, N, C, CEN = 128, 196, 768, 64
BL = B // N_CORES            # 16 local batches
D6 = C // 128                # 6 d-chunks
MB = 4 * N                   # 784 x-block columns = 4 batches
NBLK = BL // 4               # 4 blocks
ECH = [(0, 384), (384, 384)]             # e-chunks for 768
MCH = [(0, 128), (128, N - 128)]         # m-chunks per batch: 128 + 68
HALF = N * N // 2            # 19208 flat (m,i) elements per half (98 m-rows)
CH = 392                     # mixture chunk = 2 m-rows
PCW = 7 * CH                 # 2744 vct piece width (7 m-pair chunks)
NPC = HALF // PCW            # 7 pieces
P2_START = 8                 # batches >= this get z computed after the mixture


def _r(ap):
    return ap.bitcast(F32R)


def build_nc():
    nc = bacc.Bacc("TRN2", target_bir_lowering=False, debug=False)
    xt_d = nc.dram_tensor("xt", [C, BL * N], F32R, kind="ExternalInput").ap()
    kc_d = nc.dram_tensor("kc", [C, CEN], F32, kind="ExternalInput").ap()
    vct_d = nc.dram_tensor("vct", [128, HALF], F32R, kind="ExternalInput").ap()
    wt_d = nc.dram_tensor("wt", [C, C], F32R, kind="ExternalInput").ap()
    pb_d = nc.dram_tensor("pb", [128, C], F32, kind="ExternalInput").ap()
    out_d = nc.dram_tensor("out", [BL * N, C], F32, kind="ExternalOutput").ap()

    with tile.TileContext(nc) as tc:
        with (
            tc.tile_pool(name="consts", bufs=1) as consts,
            tc.tile_pool(name="wtp", bufs=D6) as wtp,
            tc.tile_pool(name="kcp", bufs=D6) as kcp,
            tc.tile_pool(name="xtp", bufs=12) as xtp,
            tc.tile_pool(name="vctp", bufs=2) as vctp,
            tc.tile_pool(name="zp", bufs=8) as zp,
            tc.tile_pool(name="vtp", bufs=1) as vtp,
            tc.tile_pool(name="smallp", bufs=2) as smallp,
            tc.tile_pool(name="stagep", bufs=2) as stagep,
            tc.tile_pool(name="outp", bufs=3) as outp,
            tc.tile_pool(name="dramp", bufs=1, space="DRAM") as dramp,
            tc.tile_pool(name="ps_z", bufs=3, space="PSUM") as ps_z,
        ):
            ones = consts.tile([128, 1], F32)
            nc.vector.memset(ones[:], 1.0)
            ident = consts.tile([128, 128], F32)
            make_identity(nc, ident[:])
            pb_t = consts.tile([128, C], F32)
            nc.sync.dma_start(pb_t[:], pb_d[:])
            qt_all = consts.tile([128, D6 * BL], F32)   # per d-chunk: 16 batch sums
            vT = vtp.tile([128, BL * 2 * N], F32R)       # [m%128][b, m//128, i]

            wt_t = []
            for d in range(D6):
                t = wtp.tile([128, C], F32R, tag="wt")
                nc.sync.dma_start(t[:], wt_d[128 * d:128 * (d + 1), :])
                wt_t.append(t)
            kc_t = []
            for d in range(D6):
                t = kcp.tile([128, CEN], F32, tag="kc")
                nc.sync.dma_start(t[:], kc_d[128 * d:128 * (d + 1), :])
                kc_t.append(t)

            z_t = [None] * BL

            def z_batch(b, xb):
                """z[b] = x[b] @ W.T as (128 part m, half, 768 e), from block tiles."""
                lb = b % 4
                zt = zp.tile([128, 2 * C], F32R, tag="z")
                z_t[b] = zt
                for mi, (moff, msz) in enumerate(MCH):
                    pse = [ps_z.tile([128, 384], F32, tag="psz", name=f"psz{e}") for e in range(2)]
                    for d in range(D6):
                        lhsT = xb[d][:, lb * N + moff: lb * N + moff + msz]
                        for e, (eoff, esz) in enumerate(ECH):
                            nc.tensor.matmul(
                                pse[e][:msz, :esz],
                                lhsT=lhsT,
                                rhs=wt_t[d][:, eoff:eoff + esz],
                                start=(d == 0), stop=(d == D6 - 1),
                            )
                    for e, (eoff, esz) in enumerate(ECH):
                        nc.scalar.copy(
                            zt[:msz, mi * C + eoff: mi * C + eoff + esz],
                            pse[e][:msz, :esz],
                        )

            # ---- phase B: x loads, q reduction, z for batches < P2_START ----
            xb_t = []
            for k in range(NBLK):
                xb = []
                for d in range(D6):
                    t = xtp.tile([128, MB], F32R, tag="xb")
                    nc.sync.dma_start(t[:], xt_d[128 * d:128 * (d + 1),
                                                 MB * k: MB * (k + 1)])
                    xb.append(t)
                xb_t.append(xb)
                for d in range(D6):
                    nc.vector.tensor_reduce(
                        out=qt_all[:, d * BL + 4 * k: d * BL + 4 * k + 4],
                        in_=xb[d].bitcast(F32).rearrange("p (t n) -> p t n", n=N),
                        axis=AX.X, op=ALU.add,
                    )
                if 4 * k + 3 < P2_START:
                    for lb4 in range(4):
                        z_batch(4 * k + lb4, xb)

            # ---- phase C: attention weights ----
            with tc.tile_pool(name="ps_c", bufs=2, space="PSUM") as ps_c:
                qsq = smallp.tile([128, D6 * BL], F32, tag="qsq")
                nc.scalar.square(qsq[:], qt_all[:])
                ssqq = ps_c.tile([BL, 1], F32, tag="psc1")
                for d in range(D6):
                    nc.tensor.matmul(ssqq[:], lhsT=qsq[:, d * BL:(d + 1) * BL],
                                     rhs=ones[:], start=(d == 0), stop=(d == D6 - 1))
                rq1 = smallp.tile([BL, 1], F32, tag="rq1")
                nc.vector.reciprocal(rq1[:], ssqq[:])
                rs_q = smallp.tile([BL, 1], F32, tag="rsq")
                nc.scalar.activation(rs_q[:], rq1[:], ACT_F.Sqrt, scale=1.0 / C)

                ksq = smallp.tile([128, CEN], F32, tag="ksq")
                ssqk = ps_c.tile([CEN, 1], F32, tag="psc1")
                for d in range(D6):
                    nc.scalar.square(ksq[:], kc_t[d][:])
                    nc.tensor.matmul(ssqk[:], lhsT=ksq[:], rhs=ones[:],
                                     start=(d == 0), stop=(d == D6 - 1))
                rk1 = smallp.tile([CEN, 1], F32, tag="rk1")
                nc.vector.reciprocal(rk1[:], ssqk[:])
                rt = smallp.tile([CEN, 1], F32, tag="rt")
                nc.scalar.sqrt(rt[:], rk1[:])

                lgT = ps_c.tile([CEN, BL], F32, tag="psc1")
                for d in range(D6):
                    nc.tensor.matmul(lgT[:], lhsT=kc_t[d][:],
                                     rhs=qt_all[:, d * BL:(d + 1) * BL],
                                     start=(d == 0), stop=(d == D6 - 1))
                lgT_s = smallp.tile([CEN, BL], F32, tag="lgts")
                nc.vector.tensor_scalar_mul(lgT_s[:], lgT[:], rt[:])

                lg = ps_c.tile([BL, CEN], F32, tag="psc2")
                nc.tensor.transpose(lg[:], lgT_s[:], ident[:CEN, :CEN])
                lg2 = smallp.tile([BL, CEN], F32, tag="lg2")
                nc.vector.tensor_scalar_mul(lg2[:], lg[:], rs_q[:])

                mx = smallp.tile([BL, 1], F32, tag="mx")
                nc.vector.tensor_reduce(out=mx[:], in_=lg2[:], axis=AX.X, op=ALU.max)
                nmx = smallp.tile([BL, 1], F32, tag="nmx")
                nc.vector.tensor_scalar_mul(nmx[:], mx[:], -1.0)
                ex = smallp.tile([BL, CEN], F32, tag="ex")
                sm = smallp.tile([BL, 1], F32, tag="sm")
                nc.scalar.activation(ex[:], lg2[:], ACT_F.Exp, bias=nmx[:],
                                     accum_out=sm[:])
                rsm = smallp.tile([BL, 1], F32, tag="rsm")
                nc.vector.reciprocal(rsm[:], sm[:])
                attn = smallp.tile([BL, CEN], F32, tag="attn")
                nc.vector.tensor_scalar_mul(attn[:], ex[:], rsm[:])

                attnT_ps = ps_c.tile([CEN, BL], F32, tag="psc1")
                nc.tensor.transpose(attnT_ps[:], attn[:], ident[:BL, :BL])
                attnT = smallp.tile([128, BL], F32R, tag="attnT")
                nc.scalar.copy(attnT[:CEN, :], attnT_ps[:])
                nc.sync.dma_start(attnT[CEN:128, :], attnT[:CEN, :])

            # ---- phase D: mixture -> DRAM bounce -> vT tiles ----
            vb = dramp.tile([BL, N * N], F32R)
            with tc.tile_pool(name="ps_mix", bufs=3, space="PSUM") as ps_mix:
                for p in range(NPC):
                    vc = vctp.tile([128, PCW], F32R, tag="vct")
                    nc.sync.dma_start(vc[:], vct_d[:, PCW * p: PCW * (p + 1)])
                    for h in range(2):
                        st = stagep.tile([BL, PCW], F32R, tag="stage",
                                         name=f"st{p}_{h}")
                        for j in range(PCW // CH):
                            ps = ps_mix.tile([BL, CH], F32, tag="psmix",
                                             name=f"psm{p}_{h}_{j}")
                            nc.tensor.matmul(
                                ps[:],
                                lhsT=attnT[64 * h: 64 * h + CEN, :],
                                rhs=vc[64 * h: 64 * h + CEN,
                                       CH * j: CH * (j + 1)],
                                start=True, stop=True,
                            )
                            if h == 0:
                                nc.vector.tensor_copy(st[:, CH * j: CH * (j + 1)], ps[:])
                            else:
                                nc.scalar.copy(st[:, CH * j: CH * (j + 1)], ps[:])
                        nc.scalar.dma_start(
                            vb[:, h * HALF + PCW * p: h * HALF + PCW * (p + 1)],
                            st[:])
            for b in range(BL):
                for mi, (moff, msz) in enumerate(MCH):
                    dst = vT[:msz].rearrange("p (b t i) -> p b t i",
                                             b=BL, t=2)[:, b, mi, :]
                    src = vb[b].rearrange("(m i) -> m i", i=N)[moff:moff + msz, :]
                    nc.scalar.dma_start(dst, src)

            # ---- final matmuls for first-half batches, then deferred z ----
            def final_batch(b):
                with tc.tile_pool(name=f"ps_f{b}", bufs=4, space="PSUM") as ps_f:
                    zt = z_t[b]
                    for ioff, isz in MCH:
                        pse = [ps_f.tile([128, 384], F32, tag="psf", name=f"psf{e}") for e in range(2)]
                        for half, (_, ksz) in enumerate(MCH):
                            lhsT = vT[:ksz].rearrange(
                                "p (b t i) -> p b t i", b=BL, t=2)[
                                :, b, half, ioff:ioff + isz]
                            for e, (eoff, esz) in enumerate(ECH):
                                nc.tensor.matmul(
                                    pse[e][:isz, :esz],
                                    lhsT=lhsT,
                                    rhs=zt[:ksz, half * C + eoff:
                                             half * C + eoff + esz],
                                    start=(half == 0), stop=(half == 1),
                                )
                        ot = outp.tile([128, C], F32, tag="ot")
                        for e, (eoff, esz) in enumerate(ECH):
                            nc.vector.tensor_tensor(
                                out=ot[:isz, eoff:eoff + esz],
                                in0=pse[e][:isz, :esz],
                                in1=pb_t[:isz, eoff:eoff + esz], op=ALU.add)
                        nc.sync.dma_start(
                            out_d[b * N + ioff: b * N + ioff + isz, :],
                            ot[:isz, :])

            for b in range(P2_START):
                final_batch(b)
            for b in range(P2_START, BL):
                z_batch(b, xb_t[b // 4])
            for b in range(P2_START, BL):
                final_batch(b)

    nc.compile()
    return nc


_NC = None


def _get_nc():
    global _NC
    if _NC is None:
        _NC = build_nc()
    return _NC


def kernel(x, k_c, v_c, proj_w, proj_b):
    x = np.ascontiguousarray(x, dtype=np.float32)
    k_c = np.ascontiguousarray(k_c, dtype=np.float32)
    v_c = np.ascontiguousarray(v_c, dtype=np.float32)
    proj_w = np.ascontiguousarray(proj_w, dtype=np.float32)
    proj_b = np.ascontiguousarray(proj_b, dtype=np.float32)

    # replicated, layout-marshalled operands
    vt = v_c.transpose(0, 2, 1).reshape(CEN, N * N)      # [c, m*196+i]
    vct = np.concatenate([vt[:, :HALF], vt[:, HALF:]], axis=0)  # (128, 19208)
    vct = np.ascontiguousarray(vct)
    wt = np.ascontiguousarray(proj_w.T)                  # wt[d, e] = proj_w[e, d]
    pb = np.ascontiguousarray(np.broadcast_to(proj_b, (128, C)))

    in_maps = []
    for c in range(N_CORES):
        xs = x[BL * c: BL * (c + 1)].reshape(BL * N, C)
        xt = np.ascontiguousarray(xs.T)                  # (768, 3136)
        in_maps.append({"xt": xt, "kc": k_c, "vct": vct, "wt": wt, "pb": pb})

    nc = _get_nc()
    res = run_bass_kernel_spmd(nc, in_maps, list(range(N_CORES)))
    out = np.concatenate(
        [r["out"].reshape(BL, N, C) for r in res.results], axis=0)
    return out


# revision 5
# speedup vs baseline: 1.2750x; 1.0032x over previous
"""Trainium2 Bass kernel for nn_Domain_Attention.

Reference computation (B=128, N=196, C=768, centers=64):
    q    = x.mean(1)                                  (B, C)
    attn = softmax(l2n(q) @ l2n(k_c) * C**-0.5)       (B, 64)
    v    = einsum('bc,cnm->bnm', attn, v_c)           (B, N, N)
    out  = einsum('bnm,bmd->bnd', v, x) @ W.T + b     (B, N, C)

Strategy: data-parallel over B across 8 NeuronCores (16 batches/core),
k_c / v_c / proj weights replicated.  Per core the computation is
reassociated as out[b] = v[b] @ (x[b] @ W.T) + b so the big GEMMs keep
the contraction dim on SBUF partitions with no on-chip transposes of
large tensors; the only cross-partition move is the (16, 38416) mixture
result -> per-m-row vT tiles, done with SBUF->SBUF DMAs of 784B runs.

Matmuls run as float32r (full-rate fp32 path at moving-dim >= 256).
"""

import numpy as np

import concourse.bass as bass
import concourse.mybir as mybir
import concourse.tile as tile
from concourse import bacc
from concourse.bass_utils import run_bass_kernel_spmd
from concourse.masks import make_identity

F32 = mybir.dt.float32
F32R = mybir.dt.float32r
AX = mybir.AxisListType
ALU = mybir.AluOpType
ACT_F = mybir.ActivationFunctionType

N_CORES = 8
B, N, C, CEN = 128, 196, 768, 64
BL = B // N_CORES            # 16 local batches
D6 = C // 128                # 6 d-chunks
MB = 4 * N                   # 784 x-block columns = 4 batches
NBLK = BL // 4               # 4 blocks
ECH = [(0, 384), (384, 384)]             # e-chunks for 768
MCH = [(0, 128), (128, N - 128)]         # m-chunks per batch: 128 + 68
HALF = N * N // 2            # 19208 flat (m,i) elements per half (98 m-rows)
CH = 392                     # mixture chunk = 2 m-rows
PCW = 7 * CH                 # 2744 vct piece width (7 m-pair chunks)
NPC = HALF // PCW            # 7 pieces
P2_START = 8                 # batches >= this get z computed after the mixture


def _r(ap):
    return ap.bitcast(F32R)


def build_nc():
    nc = bacc.Bacc("TRN2", target_bir_lowering=False, debug=False)
    xt_d = nc.dram_tensor("xt", [C, BL * N], F32R, kind="ExternalInput").ap()
    kc_d = nc.dram_tensor("kc", [C, CEN], F32, kind="ExternalInput").ap()
    vct_d = nc.dram_tensor("vct", [128, HALF], F32R, kind="ExternalInput").ap()
    wt_d = nc.dram_tensor("wt", [C, C], F32R, kind="ExternalInput").ap()
    pb_d = nc.dram_tensor("pb", [128, C], F32, kind="ExternalInput").ap()
    out_d = nc.dram_tensor("out", [BL * N, C], F32, kind="ExternalOutput").ap()

    with tile.TileContext(nc) as tc:
        with (
            tc.tile_pool(name="consts", bufs=1) as consts,
            tc.tile_pool(name="wtp", bufs=D6) as wtp,
            tc.tile_pool(name="kcp", bufs=D6) as kcp,
            tc.tile_pool(name="xtp", bufs=12) as xtp,
            tc.tile_pool(name="vctp", bufs=2) as vctp,
            tc.tile_pool(name="zp", bufs=8) as zp,
            tc.tile_pool(name="vtp", bufs=1) as vtp,
            tc.tile_pool(name="smallp", bufs=2) as smallp,
            tc.tile_pool(name="stagep", bufs=2) as stagep,
            tc.tile_pool(name="outp", bufs=3) as outp,
            tc.tile_pool(name="dramp", bufs=1, space="DRAM") as dramp,
            tc.tile_pool(name="ps_z", bufs=3, space="PSUM") as ps_z,
        ):
            ones = consts.tile([128, 1], F32)
            nc.vector.memset(ones[:], 1.0)
            ident = consts.tile([128, 128], F32)
            make_identity(nc, ident[:])
            pb_t = consts.tile([128, C], F32)
            nc.sync.dma_start(pb_t[:], pb_d[:])
            qt_all = consts.tile([128, D6 * BL], F32)   # per d-chunk: 16 batch sums
            vT = vtp.tile([128, BL * 2 * N], F32R)       # [m%128][b, m//128, i]

            wt_t = []
            for d in range(D6):
                t = wtp.tile([128, C], F32R, tag="wt")
                nc.sync.dma_start(t[:], wt_d[128 * d:128 * (d + 1), :])
                wt_t.append(t)
            kc_t = []
            for d in range(D6):
                t = kcp.tile([128, CEN], F32, tag="kc")
                nc.sync.dma_start(t[:], kc_d[128 * d:128 * (d + 1), :])
                kc_t.append(t)

            z_t = [None] * BL

            def z_batch(b, xb):
                """z[b] = x[b] @ W.T as (128 part m, half, 768 e), from block tiles."""
                lb = b % 4
                zt = zp.tile([128, 2 * C], F32R, tag="z")
                z_t[b] = zt
                for mi, (moff, msz) in enumerate(MCH):
                    pse = [ps_z.tile([128, 384], F32, tag="psz", name=f"psz{e}") for e in range(2)]
                    for d in range(D6):
                        lhsT = xb[d][:, lb * N + moff: lb * N + moff + msz]
                        for e, (eoff, esz) in enumerate(ECH):
                            nc.tensor.matmul(
                                pse[e][:msz, :esz],
                                lhsT=lhsT,
                                rhs=wt_t[d][:, eoff:eoff + esz],
                                start=(d == 0), stop=(d == D6 - 1),
                            )
                    for e, (eoff, esz) in enumerate(ECH):
                        nc.scalar.copy(
                            zt[:msz, mi * C + eoff: mi * C + eoff + esz],
                            pse[e][:msz, :esz],
                        )

            # ---- phase B: x loads, q reduction, z for batches < P2_START ----
            xb_t = []
            for k in range(NBLK):
                xb = []
                for d in range(D6):
                    t = xtp.tile([128, MB], F32R, tag="xb")
                    nc.sync.dma_start(t[:], xt_d[128 * d:128 * (d + 1),
                                                 MB * k: MB * (k + 1)])
                    xb.append(t)
                xb_t.append(xb)
                for d in range(D6):
                    nc.vector.tensor_reduce(
                        out=qt_all[:, d * BL + 4 * k: d * BL + 4 * k + 4],
                        in_=xb[d].bitcast(F32).rearrange("p (t n) -> p t n", n=N),
                        axis=AX.X, op=ALU.add,
                    )
                if 4 * k + 3 < P2_START:
                    for lb4 in range(4):
                        z_batch(4 * k + lb4, xb)

            # ---- phase C: attention weights ----
            with tc.tile_pool(name="ps_c", bufs=2, space="PSUM") as ps_c:
                qsq = smallp.tile([128, D6 * BL], F32, tag="qsq")
                nc.scalar.square(qsq[:], qt_all[:])
                ssqq = ps_c.tile([BL, 1], F32, tag="psc1")
                for d in range(D6):
                    nc.tensor.matmul(ssqq[:], lhsT=qsq[:, d * BL:(d + 1) * BL],
                                     rhs=ones[:], start=(d == 0), stop=(d == D6 - 1))
                rq1 = smallp.tile([BL, 1], F32, tag="rq1")
                nc.vector.reciprocal(rq1[:], ssqq[:])
                rs_q = smallp.tile([BL, 1], F32, tag="rsq")
                nc.scalar.activation(rs_q[:], rq1[:], ACT_F.Sqrt, scale=1.0 / C)

                ksq = smallp.tile([128, CEN], F32, tag="ksq")
                ssqk = ps_c.tile([CEN, 1], F32, tag="psc1")
                for d in range(D6):
                    nc.scalar.square(ksq[:], kc_t[d][:])
                    nc.tensor.matmul(ssqk[:], lhsT=ksq[:], rhs=ones[:],
                                     start=(d == 0), stop=(d == D6 - 1))
                rk1 = smallp.tile([CEN, 1], F32, tag="rk1")
                nc.vector.reciprocal(rk1[:], ssqk[:])
                rt = smallp.tile([CEN, 1], F32, tag="rt")
                nc.scalar.sqrt(rt[:], rk1[:])

                lgT = ps_c.tile([CEN, BL], F32, tag="psc1")
                for d in range(D6):
                    nc.tensor.matmul(lgT[:], lhsT=kc_t[d][:],
                                     rhs=qt_all[:, d * BL:(d + 1) * BL],
                                     start=(d == 0), stop=(d == D6 - 1))
                lgT_s = smallp.tile([CEN, BL], F32, tag="lgts")
                nc.vector.tensor_scalar_mul(lgT_s[:], lgT[:], rt[:])

                lg = ps_c.tile([BL, CEN], F32, tag="psc2")
                nc.tensor.transpose(lg[:], lgT_s[:], ident[:CEN, :CEN])
                lg2 = smallp.tile([BL, CEN], F32, tag="lg2")
                nc.vector.tensor_scalar_mul(lg2[:], lg[:], rs_q[:])

                mx = smallp.tile([BL, 1], F32, tag="mx")
                nc.vector.tensor_reduce(out=mx[:], in_=lg2[:], axis=AX.X, op=ALU.max)
                nmx = smallp.tile([BL, 1], F32, tag="nmx")
                nc.vector.tensor_scalar_mul(nmx[:], mx[:], -1.0)
                ex = smallp.tile([BL, CEN], F32, tag="ex")
                sm = smallp.tile([BL, 1], F32, tag="sm")
                nc.scalar.activation(ex[:], lg2[:], ACT_F.Exp, bias=nmx[:],
                                     accum_out=sm[:])
                rsm = smallp.tile([BL, 1], F32, tag="rsm")
                nc.vector.reciprocal(rsm[:], sm[:])
                attn = smallp.tile([BL, CEN], F32, tag="attn")
                nc.vector.tensor_scalar_mul(attn[:], ex[:], rsm[:])

                attnT_ps = ps_c.tile([CEN, BL], F32, tag="psc1")
                nc.tensor.transpose(attnT_ps[:], attn[:], ident[:BL, :BL])
                attnT = smallp.tile([128, BL], F32R, tag="attnT")
                nc.scalar.copy(attnT[:CEN, :], attnT_ps[:])
                nc.sync.dma_start(attnT[CEN:128, :], attnT[:CEN, :])

            # ---- phase D: mixture -> DRAM bounce -> vT tiles ----
            vb = dramp.tile([BL, N * N], F32R)
            with tc.tile_pool(name="ps_mix", bufs=4, space="PSUM") as ps_mix:
                for p in range(NPC):
                    vc = vctp.tile([128, PCW], F32R, tag="vct")
                    nc.sync.dma_start(vc[:], vct_d[:, PCW * p: PCW * (p + 1)])
                    for h in range(2):
                        st = stagep.tile([BL, PCW], F32R, tag="stage",
                                         name=f"st{p}_{h}")
                        for j in range(PCW // CH):
                            ps = ps_mix.tile([BL, CH], F32, tag="psmix",
                                             name=f"psm{p}_{h}_{j}")
                            nc.tensor.matmul(
                                ps[:],
                                lhsT=attnT[64 * h: 64 * h + CEN, :],
                                rhs=vc[64 * h: 64 * h + CEN,
                                       CH * j: CH * (j + 1)],
                                start=True, stop=True,
                            )
                            if h == 0:
                                nc.vector.tensor_copy(st[:, CH * j: CH * (j + 1)], ps[:])
                            else:
                                nc.scalar.copy(st[:, CH * j: CH * (j + 1)], ps[:])
                        nc.scalar.dma_start(
                            vb[:, h * HALF + PCW * p: h * HALF + PCW * (p + 1)],
                            st[:])
            for b in range(BL):
                for mi, (moff, msz) in enumerate(MCH):
                    dst = vT[:msz].rearrange("p (b t i) -> p b t i",
                                             b=BL, t=2)[:, b, mi, :]
                    src = vb[b].rearrange("(m i) -> m i", i=N)[moff:moff + msz, :]
                    nc.sync.dma_start(dst, src)

            # ---- final matmuls for first-half batches, then deferred z ----
            def final_batch(b):
                with tc.tile_pool(name=f"ps_f{b}", bufs=4, space="PSUM") as ps_f:
                    zt = z_t[b]
                    for ioff, isz in MCH:
                        pse = [ps_f.tile([128, 384], F32, tag="psf", name=f"psf{e}") for e in range(2)]
                        for half, (_, ksz) in enumerate(MCH):
                            lhsT = vT[:ksz].rearrange(
                                "p (b t i) -> p b t i", b=BL, t=2)[
                                :, b, half, ioff:ioff + isz]
                            for e, (eoff, esz) in enumerate(ECH):
                                nc.tensor.matmul(
                                    pse[e][:isz, :esz],
                                    lhsT=lhsT,
                                    rhs=zt[:ksz, half * C + eoff:
                                             half * C + eoff + esz],
                                    start=(half == 0), stop=(half == 1),
                                )
                        ot = outp.tile([128, C], F32, tag="ot")
                        for e, (eoff, esz) in enumerate(ECH):
                            nc.vector.tensor_tensor(
                                out=ot[:isz, eoff:eoff + esz],
                                in0=pse[e][:isz, :esz],
                                in1=pb_t[:isz, eoff:eoff + esz], op=ALU.add)
                        nc.sync.dma_start(
                            out_d[b * N + ioff: b * N + ioff + isz, :],
                            ot[:isz, :])

            for b in range(P2_START):
                final_batch(b)
            for b in range(P2_START, BL):
                z_batch(b, xb_t[b // 4])
            for b in range(P2_START, BL):
                final_batch(b)

    nc.compile()
    return nc


_NC = None


def _get_nc():
    global _NC
    if _NC is None:
        _NC = build_nc()
    return _NC


def kernel(x, k_c, v_c, proj_w, proj_b):
    x = np.ascontiguousarray(x, dtype=np.float32)
    k_c = np.ascontiguousarray(k_c, dtype=np.float32)
    v_c = np.ascontiguousarray(v_c, dtype=np.float32)
    proj_w = np.ascontiguousarray(proj_w, dtype=np.float32)
    proj_b = np.ascontiguousarray(proj_b, dtype=np.float32)

    # replicated, layout-marshalled operands
    vt = v_c.transpose(0, 2, 1).reshape(CEN, N * N)      # [c, m*196+i]
    vct = np.concatenate([vt[:, :HALF], vt[:, HALF:]], axis=0)  # (128, 19208)
    vct = np.ascontiguousarray(vct)
    wt = np.ascontiguousarray(proj_w.T)                  # wt[d, e] = proj_w[e, d]
    pb = np.ascontiguousarray(np.broadcast_to(proj_b, (128, C)))

    in_maps = []
    for c in range(N_CORES):
        xs = x[BL * c: BL * (c + 1)].reshape(BL * N, C)
        xt = np.ascontiguousarray(xs.T)                  # (768, 3136)
        in_maps.append({"xt": xt, "kc": k_c, "vct": vct, "wt": wt, "pb": pb})

    nc = _get_nc()
    res = run_bass_kernel_spmd(nc, in_maps, list(range(N_CORES)))
    out = np.concatenate(
        [r["out"].reshape(BL, N, C) for r in res.results], axis=0)
    return out
